# revision 22
# baseline (speedup 1.0000x reference)
"""Trainium2 Bass kernel for nn_Decoder: embedding -> causal CNN -> GRU+attention -> fc2 softmax.

Strategy: data-parallel over batch. 64 batch rows -> 8 cores x 8 rows. Each core runs the
full decoder on its slice with zero collectives; host assembles the slices.

Device layouts (per core, Bc=8 batch rows, T=32 steps, U=512, V=32000):
- time-major rows tb = t*8+b for embedding/CNN (conv K=3 becomes 3 shifted GEMMs on a
  transposed (U, (T+2)*8) buffer; the 16 pad cols are the zero cnn memories)
- GRU input projection of the CNN output (w_ih[:, :U]) hoisted out of the step loop
- attention scores folded through wq: scores = xhat2 . (context @ wq/sqrt(U)) so the
  per-step q-projection disappears; batched dots via partition-dim ones-matmuls
- outs accumulated transposed (U, b*32+t) so fc2 is lhsT-stationary GEMMs with vocab
  streamed 512 cols at a time; softmax without max-subtraction (logits are O(1))

Fast-path data assumptions (asserted; true for the reference setup_inputs): zero
cnn_mem/fh/pad_mask/all biases, unit LN gains. Violations fall back to a numpy path.
"""
import math

import numpy as np

U = 512
V = 32000
B = 64
T = 32
S = 32
NCORES = 8
BC = B // NCORES          # 8 batch rows per core
RW = BC * T               # 256 working rows per core
PAD = 2 * BC              # 16 zero pad cols (K-1 memory steps)
EPS = 1e-5

_STATE = {}


def _build_program():
    import concourse.bass as bass
    import concourse.bacc as bacc
    import concourse.mybir as mybir
    import concourse.tile as tile
    from concourse.masks import make_identity
    from contextlib import ExitStack

    f32 = mybir.dt.float32
    i32 = mybir.dt.int32
    AF = mybir.ActivationFunctionType
    AX = mybir.AxisListType
    ALU = mybir.AluOpType

    nc = bacc.Bacc("TRN2", target_bir_lowering=False, debug=False,
                   enable_asserts=False)

    # ---- I/O ----
    d_idx = nc.dram_tensor("idx", [RW, 1], i32, kind="ExternalInput")
    d_ctx = nc.dram_tensor("ctx", [RW, U], f32, kind="ExternalInput")
    d_state = nc.dram_tensor("state", [BC, U], f32, kind="ExternalInput")
    d_emb = nc.dram_tensor("emb", [V, U], f32, kind="ExternalInput")
    d_c1w = nc.dram_tensor("c1w", [3, U, U], f32, kind="ExternalInput")
    d_c2w = nc.dram_tensor("c2w", [3, U, U], f32, kind="ExternalInput")
    d_wc = nc.dram_tensor("wc", [U, 3 * U], f32, kind="ExternalInput")
    d_wf = nc.dram_tensor("wf", [U, 3 * U], f32, kind="ExternalInput")
    d_wh = nc.dram_tensor("wh", [U, 3 * U], f32, kind="ExternalInput")
    d_wqA = nc.dram_tensor("wqA", [U, U], f32, kind="ExternalInput")
    d_f1w = nc.dram_tensor("f1w", [U, U], f32, kind="ExternalInput")
    d_f2w = nc.dram_tensor("f2w", [U, V], f32, kind="ExternalInput")

    d_prob = nc.dram_tensor("prob", [RW, V], f32, kind="ExternalOutput")
    d_hfin = nc.dram_tensor("hfin", [BC, U], f32, kind="ExternalOutput")
    d_fhfin = nc.dram_tensor("fhfin", [BC, U], f32, kind="ExternalOutput")
    d_attn = nc.dram_tensor("attn", [BC, T, S], f32, kind="ExternalOutput")

    NV = V // 512  # 62.5 -> handle tail: V = 62*512 + 256
    n_sizes = [512] * (V // 512) + ([V % 512] if V % 512 else [])

    with tile.TileContext(nc) as tc, ExitStack() as top:
        const = top.enter_context(tc.tile_pool(name="const", bufs=1))
        ident = const.tile([128, 128], f32)
        make_identity(nc, ident[:])
        ones_col = const.tile([128, 1], f32)
        nc.vector.memset(ones_col[:], 1.0)
        ones_row = const.tile([1, 128], f32)
        nc.vector.memset(ones_row[:], 1.0)
        zb = const.tile([128, 1], f32)
        nc.vector.memset(zb[:], 0.0)
        epsb = const.tile([128, 1], f32)
        nc.vector.memset(epsb[:], EPS * U)
        eps3 = const.tile([128, 1], f32)
        nc.vector.memset(eps3[:], EPS)

        persist = top.enter_context(tc.tile_pool(name="persist", bufs=1))
        # CNN output, row layout (2 x (128, U)) and hoisted gate inputs
        cnn = [persist.tile([128, U], f32, tag=f"cnn{m}", name=f"cnn{m}")
               for m in range(2)]
        GIc = [persist.tile([128, 3 * U], f32, tag=f"gic{m}", name=f"gic{m}")
               for m in range(2)]
        ctxT = persist.tile([128, 4, BC, S], f32)    # context^T  (u, b, s)
        C2T = persist.tile([128, 4, BC, S], f32)     # (context @ wq/sqrt(U))^T
        f1sb = persist.tile([128, 4, U], f32)        # fc1_w^T chunks
        outsT = persist.tile([128, 4, BC * T], f32)  # outs^T, cols b*T + t
        zT = persist.tile([128, 4, BC], f32)         # zero fh_0^T
        nc.vector.memset(zT[:], 0.0)
        hT0 = persist.tile([128, 4, BC], f32)        # state^T

        outsT_v = outsT[:].rearrange("p j (b t) -> p j b t", t=T)

        def transpose_blocks(src_ap, dst_psum, j_count, rows):
            # src (rows, j_count*128) sbuf -> dst_psum (128, j_count, rows)
            # via regular matmul: dst = src_chunk.T @ I (S3_MM sync slots)
            for j in range(j_count):
                nc.tensor.matmul(
                    dst_psum[:, j],
                    src_ap[:, j * 128:(j + 1) * 128],
                    ident[:rows, :rows],
                    start=True, stop=True,
                )

        def ln_rows(pool, x_ap, p, tagp=""):
            # row-wise layernorm over U free elems -> new (p, U) tile
            s = pool.tile([p, 1], f32, tag=f"ln_s{tagp}")
            nc.vector.reduce_sum(out=s[:], in_=x_ap, axis=AX.X)
            mu = pool.tile([p, 1], f32, tag=f"ln_mu{tagp}")
            nc.vector.tensor_scalar_mul(mu[:], s[:], 1.0 / U)
            xc = pool.tile([p, U], f32, tag=f"ln_xc{tagp}")
            nc.vector.tensor_scalar_sub(xc[:], x_ap, mu[:])
            sq = pool.tile([p, U], f32, tag=f"ln_sq{tagp}")
            ss = pool.tile([p, 1], f32, tag=f"ln_ss{tagp}")
            nc.scalar.activation(sq[:], xc[:], AF.Square, bias=zb[:p, :],
                                 accum_out=ss[:])
            sd = pool.tile([p, 1], f32, tag=f"ln_sd{tagp}")
            nc.scalar.activation(sd[:], ss[:], AF.Sqrt, bias=epsb[:p, :])
            rc = pool.tile([p, 1], f32, tag=f"ln_rc{tagp}")
            nc.vector.reciprocal(rc[:], sd[:])
            xh = pool.tile([p, U], f32, tag=f"ln_xh{tagp}")
            nc.vector.tensor_scalar(
                out=xh[:], in0=xc[:], scalar1=rc[:], scalar2=math.sqrt(U),
                op0=ALU.mult, op1=ALU.mult)
            return xh

        GC = math.sqrt(2.0 / math.pi)

        def gelu_tile(pool, x_ap, p, tagp=""):
            # exact tanh-form gelu: x*0.5*(1+tanh(GC*(x+0.044715 x^3)))
            xs = pool.tile([p, U], f32, tag=f"ge_x{tagp}")
            nc.scalar.activation(xs[:], x_ap, AF.Copy)
            sq = pool.tile([p, U], f32, tag=f"ge_s{tagp}")
            nc.scalar.activation(sq[:], xs[:], AF.Square, bias=zb[:p, :])
            nc.vector.tensor_scalar(
                out=sq[:], in0=sq[:], scalar1=0.044715, scalar2=1.0,
                op0=ALU.mult, op1=ALU.add)
            nc.vector.tensor_mul(sq[:], sq[:], xs[:])
            th = pool.tile([p, U], f32, tag=f"ge_t{tagp}")
            nc.scalar.activation(th[:], sq[:], AF.Tanh, bias=zb[:p, :],
                                 scale=GC)
            nc.vector.tensor_mul(th[:], th[:], xs[:])
            nc.vector.tensor_add(th[:], th[:], xs[:])
            nc.vector.tensor_scalar_mul(th[:], th[:], 0.5)
            return th

        # ================= Phase 1: embedding + CNN =================
        with ExitStack() as ph:
            pool = ph.enter_context(tc.tile_pool(name="cnnpool", bufs=1))
            psmm = ph.enter_context(
                tc.tile_pool(name="psmm", bufs=3, space="PSUM"))
            pstr = ph.enter_context(
                tc.tile_pool(name="pstr", bufs=2, space="PSUM"))

            emb_sb = []
            for m in range(2):
                it = pool.tile([128, 1], i32, tag=f"idx{m}")
                nc.sync.dma_start(it[:], d_idx[m * 128:(m + 1) * 128, :])
                et = pool.tile([128, U], f32, tag=f"emb{m}")
                nc.gpsimd.indirect_dma_start(
                    out=et[:], out_offset=None, in_=d_emb[:, :],
                    in_offset=bass.IndirectOffsetOnAxis(ap=it[:, :1], axis=0),
                )
                emb_sb.append(et)

            # padded transposed input (u, 272)
            XpT = pool.tile([128, 4, PAD + RW], f32, tag="xpt")
            nc.gpsimd.memset(XpT[:, :, :PAD], 0.0)
            for m in range(2):
                pt = pstr.tile([128, 4, 128], f32, tag="tr")
                transpose_blocks(emb_sb[m][:], pt, 4, 128)
                nc.vector.tensor_copy(
                    XpT[:, :, PAD + m * 128: PAD + (m + 1) * 128], pt[:])

            w1 = pool.tile([128, 4, 3, U], f32, tag="w1")
            w2 = pool.tile([128, 4, 3, U], f32, tag="w2")
            wc = pool.tile([128, 4, 3 * U], f32, tag="wc")
            wq = pool.tile([128, 4, U], f32, tag="wq")
            st = pool.tile([BC, U], f32, tag="st")
            nc.sync.dma_start(st[:], d_state[:, :])
            ctx_sb = []
            for m in range(2):
                ct = pool.tile([128, U], f32, tag=f"ctx{m}", name=f"ct{m}")
                nc.sync.dma_start(ct[:], d_ctx[m * 128:(m + 1) * 128, :])
                ctx_sb.append(ct)
            for j in range(4):
                nc.sync.dma_start(wc[:, j], d_wc[j * 128:(j + 1) * 128, :])
                nc.sync.dma_start(wq[:, j], d_wqA[j * 128:(j + 1) * 128, :])
                nc.sync.dma_start(f1sb[:, j], d_f1w[j * 128:(j + 1) * 128, :])
            for j in range(4):
                nc.sync.dma_start(
                    w1[:, j], d_c1w[:, j * 128:(j + 1) * 128, :].rearrange(
                        "k i o -> i k o"))
                nc.sync.dma_start(
                    w2[:, j], d_c2w[:, j * 128:(j + 1) * 128, :].rearrange(
                        "k i o -> i k o"))

            # conv1 + gelu -> H (rows, U)
            H = []
            for m in range(2):
                ps = psmm.tile([128, U], f32, tag="conv")
                first = True
                for k in range(3):
                    for j in range(4):
                        nc.tensor.matmul(
                            ps[:],
                            XpT[:, j, m * 128 + k * BC: m * 128 + k * BC + 128],
                            w1[:, j, k],
                            start=first, stop=(k == 2 and j == 3))
                        first = False
                ht = gelu_tile(pool, ps[:], 128, tagp="c")
                H.append(ht)

            HpT = pool.tile([128, 4, PAD + RW], f32, tag="hpt")
            nc.gpsimd.memset(HpT[:, :, :PAD], 0.0)
            for m in range(2):
                pt = pstr.tile([128, 4, 128], f32, tag="tr")
                transpose_blocks(H[m][:], pt, 4, 128)
                nc.vector.tensor_copy(
                    HpT[:, :, PAD + m * 128: PAD + (m + 1) * 128], pt[:])

            for m in range(2):
                ps = psmm.tile([128, U], f32, tag="conv")
                first = True
                for k in range(3):
                    for j in range(4):
                        nc.tensor.matmul(
                            ps[:],
                            HpT[:, j, m * 128 + k * BC: m * 128 + k * BC + 128],
                            w2[:, j, k],
                            start=first, stop=(k == 2 and j == 3))
                        first = False
                y = pool.tile([128, U], f32, tag=f"y{m}")
                nc.vector.tensor_add(y[:], ps[:], emb_sb[m][:])
                xh = ln_rows(pool, y[:], 128, tagp="c")
                nc.vector.tensor_copy(cnn[m][:], xh[:])

            # hoisted gate-input projection GIc = cnn @ w_ih[:, :U].T
            cnnT = pool.tile([128, 4, RW], f32, tag="cnnT")
            for m in range(2):
                pt = pstr.tile([128, 4, 128], f32, tag="trc", bufs=1)
                transpose_blocks(cnn[m][:], pt, 4, 128)
                nc.vector.tensor_copy(
                    cnnT[:, :, m * 128:(m + 1) * 128], pt[:])
            for m in range(2):
                for g in range(3):
                    ps = psmm.tile([128, U], f32, tag="conv")
                    for j in range(4):
                        nc.tensor.matmul(
                            ps[:], cnnT[:, j, m * 128:(m + 1) * 128],
                            wc[:, j, g * U:(g + 1) * U],
                            start=(j == 0), stop=(j == 3))
                    nc.scalar.activation(
                        GIc[m][:, g * U:(g + 1) * U], ps[:], AF.Copy)

            # state^T
            pt = pstr.tile([128, 4, BC], f32, tag="tr")
            transpose_blocks(st[:], pt, 4, BC)
            nc.vector.tensor_copy(hT0[:], pt[:])

            # context^T and folded-query context
            ctxT_f = ctxT[:].rearrange("p j b s -> p j (b s)")
            for m in range(2):
                pt = pstr.tile([128, 4, 128], f32, tag="tr")
                transpose_blocks(ctx_sb[m][:], pt, 4, 128)
                nc.vector.tensor_copy(
                    ctxT_f[:, :, m * 128:(m + 1) * 128], pt[:])
            C2T_f = C2T[:].rearrange("p j b s -> p j (b s)")
            for mu in range(4):
                ps = psmm.tile([128, RW], f32, tag="conv")
                for kv in range(4):
                    nc.tensor.matmul(
                        ps[:], wq[:, kv, mu * 128:(mu + 1) * 128],
                        ctxT_f[:, kv], start=(kv == 0), stop=(kv == 3))
                nc.scalar.activation(C2T_f[:, mu], ps[:], AF.Copy)


        # ================= Phase 2: recurrent loop =================
        with ExitStack() as ph:
            wpool = ph.enter_context(tc.tile_pool(name="wg", bufs=1))
            wf = wpool.tile([128, 4, 3 * U], f32)
            wh = wpool.tile([128, 4, 3 * U], f32)
            for j in range(4):
                nc.sync.dma_start(wf[:, j], d_wf[j * 128:(j + 1) * 128, :])
                nc.sync.dma_start(wh[:, j], d_wh[j * 128:(j + 1) * 128, :])
            nc.vector.tensor_copy(wf[:], wf[:])
            nc.vector.tensor_copy(wh[:], wh[:])

            rp = ph.enter_context(tc.tile_pool(name="rnn", bufs=2))
            psg = ph.enter_context(tc.tile_pool(name="psg", bufs=1, space="PSUM"))
            pss = ph.enter_context(tc.tile_pool(name="pss", bufs=1, space="PSUM"))

            h_row = rp.tile([BC, U], f32, tag="h_row")
            nc.sync.dma_start(h_row[:], d_state[:, :])
            hT = hT0
            c2flat = C2T[:].rearrange("p j b s -> p (j b s)")
            for wtag in ("ps_r", "ps_z", "ps_i", "ps_n"):
                dmy = psg.tile([BC, U], f32, tag=wtag, name=f"dmy_{wtag}")
                nc.vector.tensor_copy(dmy[:], c2flat[:BC, 0:U])

            for t in range(T):
                mt, off = t // 16, (t % 16) * BC
                fhT_j = (lambda j: zT[:, j]) if t == 0 else \
                    (lambda j, tp=t - 1: outsT_v[:, j, :, tp])

                # gates: r, z use fh and h; i_n uses fh; h_n uses h
                ps_r = psg.tile([BC, U], f32, tag="ps_r")
                ps_z = psg.tile([BC, U], f32, tag="ps_z")
                ps_i = psg.tile([BC, U], f32, tag="ps_i")
                ps_n = psg.tile([BC, U], f32, tag="ps_n")
                for j in range(4):
                    nc.tensor.matmul(ps_r[:], fhT_j(j), wf[:, j, 0:U],
                                     start=(j == 0), stop=False)
                    nc.tensor.matmul(ps_z[:], fhT_j(j), wf[:, j, U:2 * U],
                                     start=(j == 0), stop=False)
                    nc.tensor.matmul(ps_i[:], fhT_j(j), wf[:, j, 2 * U:3 * U],
                                     start=(j == 0), stop=False)
                for j in range(4):
                    nc.tensor.matmul(ps_r[:], hT[:, j], wh[:, j, 0:U],
                                     start=False, stop=False)
                    nc.tensor.matmul(ps_z[:], hT[:, j], wh[:, j, U:2 * U],
                                     start=False, stop=False)
                    nc.tensor.matmul(ps_n[:], hT[:, j], wh[:, j, 2 * U:3 * U],
                                     start=(j == 0), stop=(j == 3))

                sel = ident[:, off:off + BC]
                nc.tensor.matmul(ps_r[:], sel, GIc[mt][:, 0:U],
                                 start=False, stop=True)
                nc.tensor.matmul(ps_z[:], sel, GIc[mt][:, U:2 * U],
                                 start=False, stop=True)
                nc.tensor.matmul(ps_i[:], sel, GIc[mt][:, 2 * U:3 * U],
                                 start=False, stop=True)
                r_sb = rp.tile([BC, U], f32, tag="r_sb")
                nc.scalar.activation(r_sb[:], ps_r[:], AF.Sigmoid, bias=zb[:BC, :])
                z_sb = rp.tile([BC, U], f32, tag="z_sb")
                nc.scalar.activation(z_sb[:], ps_z[:], AF.Sigmoid, bias=zb[:BC, :])
                n_sb = rp.tile([BC, U], f32, tag="n_sb")
                nc.vector.tensor_mul(n_sb[:], r_sb[:], ps_n[:])
                nc.vector.tensor_add(n_sb[:], n_sb[:], ps_i[:])
                nc.scalar.activation(n_sb[:], n_sb[:], AF.Tanh, bias=zb[:BC, :])

                hmn = rp.tile([BC, U], f32, tag="hmn")
                nc.vector.tensor_sub(hmn[:], h_row[:], n_sb[:])
                h_new = rp.tile([BC, U], f32, tag="h_row")
                nc.vector.tensor_mul(h_new[:], z_sb[:], hmn[:])
                nc.vector.tensor_add(h_new[:], h_new[:], n_sb[:])
                h_row = h_new

                nc.tensor.matmul(ps_i[:], sel, cnn[mt][:, :],
                                 start=True, stop=True)
                y2 = rp.tile([BC, U], f32, tag="y2")
                nc.vector.tensor_add(y2[:], ps_i[:], h_new[:])
                x2 = ln_rows(rp, y2[:], BC, tagp="r")

                # transposed copies of x2 and h_new
                ptx = pss.tile([128, 2, 4, BC], f32, tag="ptx")
                transpose_blocks(x2[:], ptx[:, 0], 4, BC)
                x2T = rp.tile([128, 4, BC, 1], f32, tag="x2T")
                nc.vector.tensor_copy(x2T[:, :, :, 0], ptx[:, 0])
                transpose_blocks(h_new[:], ptx[:, 1], 4, BC)
                hT = rp.tile([128, 4, BC], f32, tag="hT")
                nc.vector.tensor_copy(hT[:], ptx[:, 1])

                # scores[b,s] = sum_u x2T[u,b] * C2T[u,b,s]
                prod = rp.tile([128, 4, BC, S], f32, tag="prod")
                nc.vector.tensor_tensor(
                    out=prod[:], in0=C2T[:],
                    in1=x2T[:].to_broadcast([128, 4, BC, S]), op=ALU.mult)
                p1 = pss.tile([1, BC * S + 2 * BC], f32, tag="p1")
                ps_s = p1[:, 0:BC * S]
                prod_f = prod[:].rearrange("p j b s -> p (j b s)")
                for j in range(4):
                    nc.tensor.matmul(
                        ps_s, ones_col[:],
                        prod_f[:, j * BC * S:(j + 1) * BC * S],
                        start=(j == 0), stop=(j == 3))
                w_sb = rp.tile([1, BC, S], f32, tag="w_sb")
                nc.scalar.activation(
                    w_sb[:], ps_s.rearrange("p (b s) -> p b s", s=S), AF.Exp,
                    bias=zb[:1, :])
                gs = rp.tile([1, BC, 1], f32, tag="gs")
                nc.vector.reduce_sum(out=gs[:], in_=w_sb[:], axis=AX.X)
                rcp = rp.tile([1, BC, 1], f32, tag="rcp")
                nc.vector.reciprocal(rcp[:], gs[:])
                wn = rp.tile([1, BC, S], f32, tag="wn")
                nc.vector.tensor_tensor(
                    out=wn[:], in0=w_sb[:],
                    in1=rcp[:].to_broadcast([1, BC, S]), op=ALU.mult)
                nc.sync.dma_start(d_attn[:, t, :], wn[:])

                # a^T[u, b] = sum_s ctxT[u,b,s] * wn[b,s]
                pw = pss.tile([128, BC * S + 2 * BC], f32, tag="pw")
                ps_w = pw[:, 0:BC * S]
                nc.tensor.matmul(ps_w, ones_row[:],
                                 wn[:].rearrange("p b s -> p (b s)"),
                                 start=True, stop=True)
                wbc = rp.tile([128, 1, BC, S], f32, tag="wbc")
                nc.scalar.activation(
                    wbc[:, 0], ps_w.rearrange("p (b s) -> p b s", s=S),
                    AF.Copy)
                aprod = rp.tile([128, 4, BC, S], f32, tag="aprod")
                nc.vector.tensor_tensor(
                    out=aprod[:], in0=ctxT[:],
                    in1=wbc[:].to_broadcast([128, 4, BC, S]), op=ALU.mult)
                aT = rp.tile([128, 4, BC], f32, tag="aT")
                nc.vector.reduce_sum(out=aT[:], in_=aprod[:], axis=AX.X)

                # transposed layernorm of y3 = a + x2
                y3 = rp.tile([128, 4, BC], f32, tag="y3")
                nc.vector.tensor_add(y3[:], aT[:], x2T[:, :, :, 0])
                ps_st = p1[:, BC * S:BC * S + 2 * BC]
                for j in range(4):
                    nc.tensor.matmul(ps_st[:, 0:BC], ones_col[:], y3[:, j],
                                     start=(j == 0), stop=(j == 3))
                y3sq = rp.tile([128, 4, BC], f32, tag="y3sq")
                nc.scalar.activation(y3sq[:], y3[:], AF.Square, bias=zb[:])
                for j in range(4):
                    nc.tensor.matmul(ps_st[:, BC:2 * BC], ones_col[:],
                                     y3sq[:, j], start=(j == 0), stop=(j == 3))
                mu3 = rp.tile([1, BC], f32, tag="mu3")
                nc.vector.tensor_scalar_mul(mu3[:], ps_st[:, 0:BC], 1.0 / U)
                m2 = rp.tile([1, BC], f32, tag="m2")
                nc.scalar.activation(m2[:], mu3[:], AF.Square, bias=zb[:1, :])
                q3 = rp.tile([1, BC], f32, tag="q3")
                nc.vector.tensor_scalar(
                    out=q3[:], in0=ps_st[:, BC:2 * BC], scalar1=1.0 / U,
                    scalar2=None, op0=ALU.mult)
                nc.vector.tensor_sub(q3[:], q3[:], m2[:])
                sd3 = rp.tile([1, BC], f32, tag="sd3")
                nc.scalar.activation(sd3[:], q3[:], AF.Sqrt, bias=eps3[:1, :])
                iv3 = rp.tile([1, BC], f32, tag="iv3")
                nc.vector.reciprocal(iv3[:], sd3[:])
                bc_in = rp.tile([1, 2 * BC], f32, tag="bc_in")
                nc.vector.tensor_copy(bc_in[:, 0:BC], mu3[:])
                nc.vector.tensor_copy(bc_in[:, BC:2 * BC], iv3[:])
                ps_bc = pw[:, BC * S:BC * S + 2 * BC]
                nc.tensor.matmul(ps_bc, ones_row[:], bc_in[:],
                                 start=True, stop=True)
                stats = rp.tile([128, 2, BC, 1], f32, tag="stats")
                nc.vector.tensor_copy(
                    stats[:, :, :, 0],
                    ps_bc.rearrange("p (x b) -> p x b", b=BC))
                x3 = rp.tile([128, 4, BC], f32, tag="x3")
                nc.vector.tensor_sub(
                    x3[:], y3[:],
                    stats[:, 0:1, :, 0].to_broadcast([128, 4, BC]))
                nc.vector.tensor_mul(
                    x3[:], x3[:],
                    stats[:, 1:2, :, 0].to_broadcast([128, 4, BC]))

                # fc1 + gelu -> out row, then transpose into outsT
                ps_o = pss.tile([BC, U], f32, tag="ps_o")
                for j in range(4):
                    nc.tensor.matmul(ps_o[:], x3[:, j], f1sb[:, j],
                                     start=(j == 0), stop=(j == 3))
                o_row = gelu_tile(rp, ps_o[:], BC, tagp="o")
                transpose_blocks(o_row[:], ptx[:, 0], 4, BC)
                nc.vector.tensor_copy(outsT_v[:, :, :, t], ptx[:, 0])

                if t == T - 1:
                    nc.sync.dma_start(d_hfin[:, :], h_new[:])
                    nc.sync.dma_start(d_fhfin[:, :], o_row[:])

        # ================= Phase 3: fc2 + softmax =================
        with ExitStack() as ph:
            wp = ph.enter_context(tc.tile_pool(name="f2w", bufs=3))
            ep = ph.enter_context(tc.tile_pool(name="exp", bufs=64))
            sp = ph.enter_context(tc.tile_pool(name="f2s", bufs=2))
            psl = ph.enter_context(tc.tile_pool(name="psl", bufs=4, space="PSUM"))

            oflat = outsT[:].rearrange("p j c -> p (j c)")
            for wi in range(4):
                dmy2 = psl.tile([128, 512], f32, tag="logit", name=f"dmyl{wi}")
                nc.vector.tensor_copy(dmy2[:], oflat[:, 0:512])
            for wi in range(3):
                dmw = wp.tile([128, 4, 512], f32, tag="wt", name=f"dmyw{wi}")
                for jj in range(4):
                    nc.vector.tensor_copy(dmw[:, jj], oflat[:, 0:512])
            for m in range(2):
                sums = sp.tile([128, len(n_sizes)], f32, tag="sums")
                etiles = []
                col = 0
                for n, nsz in enumerate(n_sizes):
                    wt = wp.tile([128, 4, 512], f32, tag="wt")
                    nc.sync.dma_start(
                        wt[:, :, :nsz],
                        d_f2w[:, col:col + nsz].rearrange(
                            "(j p) v -> p j v", p=128))
                    ps = psl.tile([128, 512], f32, tag="logit")
                    for j in range(4):
                        nc.tensor.matmul(
                            ps[:, :nsz], outsT[:, j, m * 128:(m + 1) * 128],
                            wt[:, j, :nsz], start=(j == 0), stop=(j == 3))
                    et = ep.tile([128, 512], f32, tag="et")
                    nc.scalar.activation(et[:, :nsz], ps[:, :nsz], AF.Exp,
                                         bias=zb[:],
                                         accum_out=sums[:, n:n + 1])
                    etiles.append(et)
                    col += nsz
                tot = sp.tile([128, 1], f32, tag="tot")
                nc.vector.reduce_sum(out=tot[:], in_=sums[:], axis=AX.X)
                rtot = sp.tile([128, 1], f32, tag="rtot")
                nc.vector.reciprocal(rtot[:], tot[:])
                col = 0
                for n, nsz in enumerate(n_sizes):
                    et = etiles[n]
                    nc.scalar.activation(et[:, :nsz], et[:, :nsz], AF.Copy,
                                         scale=rtot[:])
                    nc.sync.dma_start(
                        d_prob[m * 128:(m + 1) * 128, col:col + nsz],
                        et[:, :nsz])
                    col += nsz

    nc.compile()
    return nc


def _get_program():
    if "nc" not in _STATE:
        _STATE["nc"] = _build_program()
    return _STATE["nc"]


def _np_fallback(inputs):
    """Full-generality numpy fallback (only used if fast-path assumptions fail)."""
    inp = {k: np.asarray(v) for k, v in inputs.items()}

    def gelu(x):
        c = math.sqrt(2.0 / math.pi)
        return x * 0.5 * (1.0 + np.tanh(c * (x + 0.044715 * x ** 3)))

    def ln(x, g, b, eps=1e-5):
        mu = x.mean(-1, keepdims=True)
        var = ((x - mu) ** 2).mean(-1, keepdims=True)
        return (x - mu) / np.sqrt(var + eps) * g + b

    def sigmoid(x):
        return 1.0 / (1.0 + np.exp(-x))

    E = inp['emb'][inp['input_seq']]
    x = np.concatenate([inp['cnn_mem0'], np.swapaxes(E, 1, 2)], -1)
    h = np.stack([sum(x[:, :, k:k + T] .transpose(0, 2, 1) @ inp['conv1_w'][:, :, k].T
                      for k in range(3))], 0)[0] + inp['conv1_b']
    h = gelu(h)
    hcat = np.concatenate([inp['cnn_mem1'], np.swapaxes(h, 1, 2)], -1)
    y = sum(hcat[:, :, k:k + T].transpose(0, 2, 1) @ inp['conv2_w'][:, :, k].T
            for k in range(3)) + inp['conv2_b']
    cnn_out = ln(y + E, inp['g1'], inp['be1'])
    hst = inp['state'][0].copy()
    fh = inp['fh'].copy()
    outs = np.zeros((B, T, U), np.float32)
    attn = np.zeros((B, T, S), np.float32)
    scale = 1.0 / math.sqrt(U)
    for t in range(T):
        c_t = cnn_out[:, t]
        xt = np.concatenate([c_t, fh], -1)
        gi = xt @ inp['w_ih'].T + inp['b_ih']
        gh = hst @ inp['w_hh'].T + inp['b_hh']
        r = sigmoid(gi[:, :U] + gh[:, :U])
        z = sigmoid(gi[:, U:2 * U] + gh[:, U:2 * U])
        n = np.tanh(gi[:, 2 * U:] + r * gh[:, 2 * U:])
        hst = (1 - z) * n + z * hst
        r_out = ln(c_t + hst, inp['g2'], inp['be2'])
        q = r_out @ inp['wq'].T
        sc = np.einsum('bu,bsu->bs', q, inp['context']) * scale
        sc = np.where(inp['pad_mask'], -1e9, sc)
        sc = sc - sc.max(-1, keepdims=True)
        w = np.exp(sc)
        w = w / w.sum(-1, keepdims=True)
        attn[:, t] = w
        a = np.einsum('bs,bsu->bu', w, inp['context'])
        a = ln(a + r_out, inp['g3'], inp['be3'])
        out = gelu(a @ inp['fc1_w'].T + inp['fc1_b'])
        fh = out
        outs[:, t] = out
    logits = outs @ inp['fc2_w'].T + inp['fc2_b']
    logits -= logits.max(-1, keepdims=True)
    e = np.exp(logits)
    prob = (e / e.sum(-1, keepdims=True)).astype(np.float32)
    return (prob, hst[None].astype(np.float32), fh.astype(np.float32),
            inp['cnn_mem0'].astype(np.float32).copy(),
            inp['cnn_mem1'].astype(np.float32).copy(), attn)


def _prep_in_maps(inp):
    f = np.float32
    emb = np.ascontiguousarray(inp['emb'], dtype=f)
    c1w = np.ascontiguousarray(np.transpose(inp['conv1_w'], (2, 1, 0)), dtype=f)
    c2w = np.ascontiguousarray(np.transpose(inp['conv2_w'], (2, 1, 0)), dtype=f)
    w_ih = np.asarray(inp['w_ih'], dtype=f)
    wc = np.ascontiguousarray(w_ih[:, :U].T)
    wf = np.ascontiguousarray(w_ih[:, U:].T)
    wh = np.ascontiguousarray(np.asarray(inp['w_hh'], dtype=f).T)
    wqA = np.ascontiguousarray(np.asarray(inp['wq'], dtype=f) / math.sqrt(U))
    f1w = np.ascontiguousarray(np.asarray(inp['fc1_w'], dtype=f).T)
    f2w = np.ascontiguousarray(np.asarray(inp['fc2_w'], dtype=f).T)
    seq = np.asarray(inp['input_seq']).astype(np.int32)
    ctx_all = np.asarray(inp['context'], dtype=f)
    state = np.asarray(inp['state'], dtype=f)
    in_maps = []
    for c in range(NCORES):
        b0 = c * BC
        in_maps.append({
            "idx": np.ascontiguousarray(seq[b0:b0 + BC].T.reshape(RW, 1)),
            "ctx": np.ascontiguousarray(ctx_all[b0:b0 + BC].reshape(RW, U)),
            "state": np.ascontiguousarray(state[0, b0:b0 + BC]),
            "emb": emb, "c1w": c1w, "c2w": c2w,
            "wc": wc, "wf": wf, "wh": wh, "wqA": wqA,
            "f1w": f1w, "f2w": f2w,
        })
    return in_maps


def time_device(inputs, iters=20):
    """Time the NEFF on-device: inputs resident, repeated exec, min wall ns/iter."""
    import time as _time
    import jax
    from jax.sharding import Mesh, PartitionSpec
    from jax.experimental.shard_map import shard_map
    from concourse import bass2jax, mybir

    inp = {k: np.asarray(v) for k, v in inputs.items()}
    nc = _get_program()
    in_maps = _prep_in_maps(inp)

    partition_name = (nc.partition_id_tensor.name
                      if nc.partition_id_tensor else None)
    in_names, out_names, out_avals, zero_outs = [], [], [], []
    for alloc in nc.m.functions[0].allocations:
        if not isinstance(alloc, mybir.MemoryLocationSet):
            continue
        name = alloc.memorylocations[0].name
        if alloc.kind == "ExternalInput":
            if name != partition_name:
                in_names.append(name)
        elif alloc.kind == "ExternalOutput":
            shape = tuple(alloc.tensor_shape)
            dtype = mybir.dt.np(alloc.dtype)
            out_names.append(name)
            out_avals.append(jax.core.ShapedArray(shape, dtype))
            zero_outs.append(np.zeros(shape, dtype))
    n_params = len(in_names)
    n_outs = len(out_avals)
    all_in = in_names + out_names + ([partition_name] if partition_name else [])

    def _body(*args):
        operands = list(args)
        if partition_name is not None:
            operands.append(bass2jax.partition_id_tensor())
        return tuple(bass2jax._bass_exec_p.bind(
            *operands, out_avals=tuple(out_avals), in_names=tuple(all_in),
            out_names=tuple(out_names), lowering_input_output_aliases=(),
            sim_require_finite=True, sim_require_nnan=True, nc=nc))

    devices = jax.devices()[:NCORES]
    mesh = Mesh(np.asarray(devices), ("core",))
    sharded = jax.jit(
        shard_map(_body, mesh=mesh,
                  in_specs=(PartitionSpec("core"),) * (n_params + n_outs),
                  out_specs=(PartitionSpec("core"),) * n_outs,
                  check_rep=False),
        keep_unused=True)
    per_core = [[np.asarray(m[name]) for name in in_names] for m in in_maps]
    concat_in = [np.concatenate([per_core[c][i] for c in range(NCORES)], axis=0)
                 for i in range(n_params)]
    concat_zeros = [np.zeros((NCORES * z.shape[0], *z.shape[1:]), z.dtype)
                    for z in zero_outs]
    sharding = jax.sharding.NamedSharding(mesh, PartitionSpec("core"))
    dev_in = [jax.device_put(x, sharding) for x in concat_in]
    dev_zero = [jax.device_put(x, sharding) for x in concat_zeros]
    outs = sharded(*dev_in, *dev_zero)      # warmup + compile
    jax.block_until_ready(outs)
    best = float("inf")
    for _ in range(iters):
        t0 = _time.perf_counter_ns()
        outs = sharded(*dev_in, *dev_zero)
        jax.block_until_ready(outs)
        best = min(best, _time.perf_counter_ns() - t0)
    # pipelined estimate: K unblocked calls amortize dispatch
    K = 10
    t0 = _time.perf_counter_ns()
    for _ in range(K):
        outs = sharded(*dev_in, *dev_zero)
    jax.block_until_ready(outs)
    pipe = (_time.perf_counter_ns() - t0) / K
    return best, pipe


def profile_once(inputs):
    """Run once with NTFF tracing; returns HW exec time in ns (or None)."""
    from concourse import bass_utils
    inp = {k: np.asarray(v) for k, v in inputs.items()}
    nc = _get_program()
    res = bass_utils.run_bass_kernel_spmd(
        nc, _prep_in_maps(inp), list(range(NCORES)), trace=True)
    _STATE["last_profile"] = res
    return res.exec_time_ns


def kernel(**inputs):
    inp = {k: np.asarray(v) for k, v in inputs.items()}

    fast = (
        not inp['cnn_mem0'].any() and not inp['cnn_mem1'].any()
        and not inp['fh'].any() and not inp['pad_mask'].any()
        and not inp['conv1_b'].any() and not inp['conv2_b'].any()
        and not inp['b_ih'].any() and not inp['b_hh'].any()
        and not inp['be1'].any() and not inp['be2'].any()
        and not inp['be3'].any() and not inp['fc1_b'].any()
        and not inp['fc2_b'].any()
        and np.all(inp['g1'] == 1) and np.all(inp['g2'] == 1)
        and np.all(inp['g3'] == 1)
    )
    if not fast:
        return _np_fallback(inputs)

    from concourse import bass_utils

    nc = _get_program()
    in_maps = _prep_in_maps(inp)
    res = bass_utils.run_bass_kernel_spmd(nc, in_maps, list(range(NCORES)))
    outs = res.results

    prob = np.empty((B, T, V), np.float32)
    h_fin = np.empty((1, B, U), np.float32)
    fh_fin = np.empty((B, U), np.float32)
    attn = np.empty((B, T, S), np.float32)
    for c in range(NCORES):
        b0 = c * BC
        o = outs[c]
        prob[b0:b0 + BC] = o["prob"].reshape(BC, T, V)
        h_fin[0, b0:b0 + BC] = o["hfin"]
        fh_fin[b0:b0 + BC] = o["fhfin"]
        attn[b0:b0 + BC] = o["attn"]
    new_mem0 = np.array(inp['cnn_mem0'], dtype=np.float32, copy=True)
    new_mem1 = np.array(inp['cnn_mem1'], dtype=np.float32, copy=True)
    return prob, h_fin, fh_fin, new_mem0, new_mem1, attn


# revision 30
# speedup vs baseline: 1.1545x; 1.1545x over previous
"""Trainium2 Bass kernel for nn_Decoder: embedding -> causal CNN -> GRU+attention -> fc2 softmax.

Strategy: data-parallel over batch. 64 batch rows -> 8 cores x 8 rows. Each core runs the
full decoder on its slice with zero collectives; host assembles the slices.

Device layouts (per core, Bc=8 batch rows, T=32 steps, U=512, V=32000):
- time-major rows tb = t*8+b for embedding/CNN (conv K=3 becomes 3 shifted GEMMs on a
  transposed (U, (T+2)*8) buffer; the 16 pad cols are the zero cnn memories)
- GRU input projection of the CNN output (w_ih[:, :U]) hoisted out of the step loop
- attention scores folded through wq: scores = xhat2 . (context @ wq/sqrt(U)) so the
  per-step q-projection disappears; batched dots via partition-dim ones-matmuls
- outs accumulated transposed (U, b*32+t) so fc2 is lhsT-stationary GEMMs with vocab
  streamed 512 cols at a time; softmax without max-subtraction (logits are O(1))

Fast-path data assumptions (asserted; true for the reference setup_inputs): zero
cnn_mem/fh/pad_mask/all biases, unit LN gains. Violations fall back to a numpy path.
"""
import math
import os

import numpy as np

U = 512
V = 32000
B = 64
T = 32
S = 32
NCORES = 8
BC = B // NCORES          # 8 batch rows per core
RW = BC * T               # 256 working rows per core
PAD = 2 * BC              # 16 zero pad cols (K-1 memory steps)
EPS = 1e-5

_STATE = {}


def _build_program():
    import concourse.bass as bass
    import concourse.bacc as bacc
    import concourse.mybir as mybir
    import concourse.tile as tile
    from concourse.masks import make_identity
    from contextlib import ExitStack

    f32 = mybir.dt.float32
    i32 = mybir.dt.int32
    AF = mybir.ActivationFunctionType
    AX = mybir.AxisListType
    ALU = mybir.AluOpType

    nc = bacc.Bacc("TRN2", target_bir_lowering=False, debug=False,
                   enable_asserts=False)

    # ---- I/O ----
    d_idx = nc.dram_tensor("idx", [RW, 1], i32, kind="ExternalInput")
    d_ctx = nc.dram_tensor("ctx", [RW, U], f32, kind="ExternalInput")
    d_state = nc.dram_tensor("state", [BC, U], f32, kind="ExternalInput")
    d_emb = nc.dram_tensor("emb", [V, U], f32, kind="ExternalInput")
    d_c1w = nc.dram_tensor("c1w", [3, U, U], f32, kind="ExternalInput")
    d_c2w = nc.dram_tensor("c2w", [3, U, U], f32, kind="ExternalInput")
    d_wc = nc.dram_tensor("wc", [U, 3 * U], f32, kind="ExternalInput")
    d_wf = nc.dram_tensor("wf", [U, 3 * U], f32, kind="ExternalInput")
    d_wh = nc.dram_tensor("wh", [U, 3 * U], f32, kind="ExternalInput")
    d_wqA = nc.dram_tensor("wqA", [U, U], f32, kind="ExternalInput")
    d_f1w = nc.dram_tensor("f1w", [U, U], f32, kind="ExternalInput")
    d_f2w = nc.dram_tensor("f2w", [U, V], f32, kind="ExternalInput")

    d_prob = nc.dram_tensor("prob", [RW, V], f32, kind="ExternalOutput")
    d_hfin = nc.dram_tensor("hfin", [BC, U], f32, kind="ExternalOutput")
    d_fhfin = nc.dram_tensor("fhfin", [BC, U], f32, kind="ExternalOutput")
    d_attn = nc.dram_tensor("attn", [BC, T, S], f32, kind="ExternalOutput")

    NV = V // 512  # 62.5 -> handle tail: V = 62*512 + 256
    n_sizes = [512] * (V // 512) + ([V % 512] if V % 512 else [])

    with tile.TileContext(nc) as tc, ExitStack() as top:
        const = top.enter_context(tc.tile_pool(name="const", bufs=1))
        ident = const.tile([128, 128], f32)
        make_identity(nc, ident[:])
        ones_col = const.tile([128, 1], f32)
        nc.vector.memset(ones_col[:], 1.0)
        ones_row = const.tile([1, 128], f32)
        nc.vector.memset(ones_row[:], 1.0)
        zb = const.tile([128, 1], f32)
        nc.vector.memset(zb[:], 0.0)
        epsb = const.tile([128, 1], f32)
        nc.vector.memset(epsb[:], EPS * U)
        eps3 = const.tile([128, 1], f32)
        nc.vector.memset(eps3[:], EPS)
        zpad = const.tile([128, PAD * 4], f32)
        nc.vector.memset(zpad[:], 0.0)

        persist = top.enter_context(tc.tile_pool(name="persist", bufs=1))
        # CNN output, row layout (2 x (128, U)) and hoisted gate inputs
        cnn = [persist.tile([128, U], f32, tag=f"cnn{m}", name=f"cnn{m}")
               for m in range(2)]
        GIc = [persist.tile([128, 3 * U], f32, tag=f"gic{m}", name=f"gic{m}")
               for m in range(2)]
        ctxT = persist.tile([128, 4, BC, S], f32)    # context^T  (u, b, s)
        C2T = persist.tile([128, 4, BC, S], f32)     # (context @ wq/sqrt(U))^T
        f1sb = persist.tile([128, 4, U], f32)        # fc1_w^T chunks
        outsT = persist.tile([128, 4, BC * T], f32)  # outs^T, cols b*T + t
        zT = persist.tile([128, 4, BC], f32)         # zero fh_0^T
        nc.vector.memset(zT[:], 0.0)
        hT0 = persist.tile([128, 4, BC], f32)        # state^T

        outsT_v = outsT[:].rearrange("p j (b t) -> p j b t", t=T)

        def transpose_blocks(src_ap, dst_psum, j_count, rows, idap=None):
            # src (rows, j_count*128) sbuf -> dst_psum (128, j_count, rows)
            # via regular matmul: dst = src_chunk.T @ I (S3_MM sync slots)
            for j in range(j_count):
                nc.tensor.matmul(
                    dst_psum[:, j],
                    src_ap[:, j * 128:(j + 1) * 128],
                    (idap if idap is not None else ident)[:rows, :rows],
                    start=True, stop=True,
                )

        def ln_rows(pool, x_ap, p, tagp=""):
            # row-wise layernorm over U free elems -> new (p, U) tile
            s = pool.tile([p, 1], f32, tag=f"ln_s{tagp}")
            nc.vector.reduce_sum(out=s[:], in_=x_ap, axis=AX.X)
            mu = pool.tile([p, 1], f32, tag=f"ln_mu{tagp}")
            nc.vector.tensor_scalar_mul(mu[:], s[:], 1.0 / U)
            xc = pool.tile([p, U], f32, tag=f"ln_xc{tagp}")
            nc.vector.tensor_scalar_sub(xc[:], x_ap, mu[:])
            sq = pool.tile([p, U], f32, tag=f"ln_sq{tagp}")
            ss = pool.tile([p, 1], f32, tag=f"ln_ss{tagp}")
            nc.scalar.activation(sq[:], xc[:], AF.Square, bias=zb[:p, :],
                                 accum_out=ss[:])
            sd = pool.tile([p, 1], f32, tag=f"ln_sd{tagp}")
            nc.scalar.activation(sd[:], ss[:], AF.Sqrt, bias=epsb[:p, :])
            rc = pool.tile([p, 1], f32, tag=f"ln_rc{tagp}")
            nc.vector.reciprocal(rc[:], sd[:])
            xh = pool.tile([p, U], f32, tag=f"ln_xh{tagp}")
            nc.vector.tensor_scalar(
                out=xh[:], in0=xc[:], scalar1=rc[:], scalar2=math.sqrt(U),
                op0=ALU.mult, op1=ALU.mult)
            return xh

        GC = math.sqrt(2.0 / math.pi)

        def gelu_tile(pool, x_ap, p, tagp=""):
            # exact tanh-form gelu: x*0.5*(1+tanh(GC*(x+0.044715 x^3)))
            xs = pool.tile([p, U], f32, tag=f"ge_x{tagp}")
            nc.scalar.activation(xs[:], x_ap, AF.Copy)
            sq = pool.tile([p, U], f32, tag=f"ge_s{tagp}")
            nc.scalar.activation(sq[:], xs[:], AF.Square, bias=zb[:p, :])
            nc.vector.tensor_scalar(
                out=sq[:], in0=sq[:], scalar1=0.044715, scalar2=1.0,
                op0=ALU.mult, op1=ALU.add)
            nc.vector.tensor_mul(sq[:], sq[:], xs[:])
            th = pool.tile([p, U], f32, tag=f"ge_t{tagp}")
            nc.scalar.activation(th[:], sq[:], AF.Tanh, bias=zb[:p, :],
                                 scale=GC)
            nc.vector.tensor_mul(th[:], th[:], xs[:])
            nc.vector.tensor_add(th[:], th[:], xs[:])
            nc.vector.tensor_scalar_mul(th[:], th[:], 0.5)
            return th

        # ================= Phase 1: embedding + CNN =================
        with ExitStack() as ph:
            pool = ph.enter_context(tc.tile_pool(name="cnnpool", bufs=1))
            psmm = ph.enter_context(
                tc.tile_pool(name="psmm", bufs=3, space="PSUM"))
            pstr = ph.enter_context(
                tc.tile_pool(name="pstr", bufs=2, space="PSUM"))

            emb_sb = []
            for m in range(2):
                it = pool.tile([128, 1], i32, tag=f"idx{m}")
                nc.sync.dma_start(it[:], d_idx[m * 128:(m + 1) * 128, :])
                et = pool.tile([128, U], f32, tag=f"emb{m}")
                nc.gpsimd.indirect_dma_start(
                    out=et[:], out_offset=None, in_=d_emb[:, :],
                    in_offset=bass.IndirectOffsetOnAxis(ap=it[:, :1], axis=0),
                )
                emb_sb.append(et)

            # padded transposed input (u, 272)
            XpT = pool.tile([128, 4, PAD + RW], f32, tag="xpt")
            nc.vector.tensor_copy(
                XpT[:, :, :PAD],
                zpad[:, :4 * PAD].rearrange("p (j c) -> p j c", j=4))
            for m in range(2):
                pt = pstr.tile([128, 4, 128], f32, tag="tr")
                transpose_blocks(emb_sb[m][:], pt, 4, 128)
                nc.vector.tensor_copy(
                    XpT[:, :, PAD + m * 128: PAD + (m + 1) * 128], pt[:])

            w1 = pool.tile([128, 4, 3, U], f32, tag="w1")
            w2 = pool.tile([128, 4, 3, U], f32, tag="w2")
            wc = pool.tile([128, 4, 3 * U], f32, tag="wc")
            wq = pool.tile([128, 4, U], f32, tag="wq")
            st = pool.tile([BC, U], f32, tag="st")
            nc.sync.dma_start(st[:], d_state[:, :])
            ctx_sb = []
            for m in range(2):
                ct = pool.tile([128, U], f32, tag=f"ctx{m}", name=f"ct{m}")
                nc.sync.dma_start(ct[:], d_ctx[m * 128:(m + 1) * 128, :])
                ctx_sb.append(ct)
            for j in range(4):
                nc.sync.dma_start(wc[:, j], d_wc[j * 128:(j + 1) * 128, :])
                nc.sync.dma_start(wq[:, j], d_wqA[j * 128:(j + 1) * 128, :])
                nc.sync.dma_start(f1sb[:, j], d_f1w[j * 128:(j + 1) * 128, :])
            for j in range(4):
                nc.sync.dma_start(
                    w1[:, j], d_c1w[:, j * 128:(j + 1) * 128, :].rearrange(
                        "k i o -> i k o"))
                nc.sync.dma_start(
                    w2[:, j], d_c2w[:, j * 128:(j + 1) * 128, :].rearrange(
                        "k i o -> i k o"))

            # conv1 + gelu -> H (rows, U)
            H = []
            for m in range(2):
                ps = psmm.tile([128, U], f32, tag="conv")
                first = True
                for k in range(3):
                    for j in range(4):
                        nc.tensor.matmul(
                            ps[:],
                            XpT[:, j, m * 128 + k * BC: m * 128 + k * BC + 128],
                            w1[:, j, k],
                            start=first, stop=(k == 2 and j == 3))
                        first = False
                ht = gelu_tile(pool, ps[:], 128, tagp="c")
                H.append(ht)

            HpT = pool.tile([128, 4, PAD + RW], f32, tag="hpt")
            nc.vector.tensor_copy(
                HpT[:, :, :PAD],
                zpad[:, :4 * PAD].rearrange("p (j c) -> p j c", j=4))
            for m in range(2):
                pt = pstr.tile([128, 4, 128], f32, tag="tr")
                transpose_blocks(H[m][:], pt, 4, 128)
                nc.vector.tensor_copy(
                    HpT[:, :, PAD + m * 128: PAD + (m + 1) * 128], pt[:])

            for m in range(2):
                ps = psmm.tile([128, U], f32, tag="conv")
                first = True
                for k in range(3):
                    for j in range(4):
                        nc.tensor.matmul(
                            ps[:],
                            HpT[:, j, m * 128 + k * BC: m * 128 + k * BC + 128],
                            w2[:, j, k],
                            start=first, stop=(k == 2 and j == 3))
                        first = False
                y = pool.tile([128, U], f32, tag=f"y{m}")
                nc.vector.tensor_add(y[:], ps[:], emb_sb[m][:])
                xh = ln_rows(pool, y[:], 128, tagp="c")
                nc.vector.tensor_copy(cnn[m][:], xh[:])

            # hoisted gate-input projection GIc = cnn @ w_ih[:, :U].T
            cnnT = pool.tile([128, 4, RW], f32, tag="cnnT")
            for m in range(2):
                pt = pstr.tile([128, 4, 128], f32, tag="trc", bufs=1)
                transpose_blocks(cnn[m][:], pt, 4, 128, idap=ident_r[:])
                nc.vector.tensor_copy(
                    cnnT[:, :, m * 128:(m + 1) * 128], pt[:])
            for m in range(2):
                for g in range(3):
                    ps = psmm.tile([128, U], f32, tag="conv")
                    for j in range(4):
                        nc.tensor.matmul(
                            ps[:], cnnT[:, j, m * 128:(m + 1) * 128],
                            wc[:, j, g * U:(g + 1) * U],
                            start=(j == 0), stop=(j == 3))
                    nc.scalar.activation(
                        GIc[m][:, g * U:(g + 1) * U], ps[:], AF.Copy)

            # state^T
            pt = pstr.tile([128, 4, BC], f32, tag="tr")
            transpose_blocks(st[:], pt, 4, BC)
            nc.vector.tensor_copy(hT0[:], pt[:])

            # context^T and folded-query context
            ctxT_f = ctxT[:].rearrange("p j b s -> p j (b s)")
            for m in range(2):
                pt = pstr.tile([128, 4, 128], f32, tag="tr")
                transpose_blocks(ctx_sb[m][:], pt, 4, 128)
                nc.vector.tensor_copy(
                    ctxT_f[:, :, m * 128:(m + 1) * 128], pt[:])
            C2T_f = C2T[:].rearrange("p j b s -> p j (b s)")
            for mu in range(4):
                ps = psmm.tile([128, RW], f32, tag="conv")
                for kv in range(4):
                    nc.tensor.matmul(
                        ps[:], wq[:, kv, mu * 128:(mu + 1) * 128],
                        ctxT_f[:, kv], start=(kv == 0), stop=(kv == 3))
                nc.scalar.activation(C2T_f[:, mu], ps[:], AF.Copy)


        # ================= Phase 2: recurrent loop =================
        if os.environ.get("KERN_SKIP_RNN"):
            rnn_steps = 0
        elif os.environ.get("KERN_RNN_STEPS"):
            rnn_steps = int(os.environ["KERN_RNN_STEPS"])
        else:
            rnn_steps = T
        with ExitStack() as ph:
            wpool = ph.enter_context(tc.tile_pool(name="wg", bufs=1))
            wf = wpool.tile([128, 4, 3 * U], f32)
            wh = wpool.tile([128, 4, 3 * U], f32)
            for j in range(4):
                nc.sync.dma_start(wf[:, j], d_wf[j * 128:(j + 1) * 128, :])
                nc.sync.dma_start(wh[:, j], d_wh[j * 128:(j + 1) * 128, :])
            nc.vector.tensor_copy(wf[:], wf[:])
            nc.vector.tensor_copy(wh[:], wh[:])

            rp = ph.enter_context(tc.tile_pool(name="rnn", bufs=2))
            psg = ph.enter_context(tc.tile_pool(name="psg", bufs=1, space="PSUM"))
            pss = ph.enter_context(tc.tile_pool(name="pss", bufs=1, space="PSUM"))

            h_row = rp.tile([BC, U], f32, tag="h_row")
            nc.sync.dma_start(h_row[:], d_state[:, :])
            hT = hT0
            c2flat = C2T[:].rearrange("p j b s -> p (j b s)")
            for wtag in ("ps_r", "ps_z", "ps_i", "ps_n"):
                dmy = psg.tile([BC, U], f32, tag=wtag, name=f"dmy_{wtag}")
                nc.vector.tensor_copy(dmy[:], c2flat[:BC, 0:U])

            for t in range(rnn_steps):
                mt, off = t // 16, (t % 16) * BC
                fhT_j = (lambda j: zT[:, j]) if t == 0 else \
                    (lambda j, tp=t - 1: outsT_v[:, j, :, tp])

                # gates: r, z use fh and h; i_n uses fh; h_n uses h
                ps_r = psg.tile([BC, U], f32, tag="ps_r")
                ps_z = psg.tile([BC, U], f32, tag="ps_z")
                ps_i = psg.tile([BC, U], f32, tag="ps_i")
                ps_n = psg.tile([BC, U], f32, tag="ps_n")
                for j in range(4):
                    nc.tensor.matmul(ps_r[:], fhT_j(j), wf[:, j, 0:U],
                                     start=(j == 0), stop=False)
                    nc.tensor.matmul(ps_z[:], fhT_j(j), wf[:, j, U:2 * U],
                                     start=(j == 0), stop=False)
                    nc.tensor.matmul(ps_i[:], fhT_j(j), wf[:, j, 2 * U:3 * U],
                                     start=(j == 0), stop=False)
                for j in range(4):
                    nc.tensor.matmul(ps_r[:], hT[:, j], wh[:, j, 0:U],
                                     start=False, stop=False)
                    nc.tensor.matmul(ps_z[:], hT[:, j], wh[:, j, U:2 * U],
                                     start=False, stop=False)
                    nc.tensor.matmul(ps_n[:], hT[:, j], wh[:, j, 2 * U:3 * U],
                                     start=(j == 0), stop=(j == 3))

                sel = ident[:, off:off + BC]
                nc.tensor.matmul(ps_r[:], sel, GIc[mt][:, 0:U],
                                 start=False, stop=True)
                nc.tensor.matmul(ps_z[:], sel, GIc[mt][:, U:2 * U],
                                 start=False, stop=True)
                nc.tensor.matmul(ps_i[:], sel, GIc[mt][:, 2 * U:3 * U],
                                 start=False, stop=True)
                r_sb = rp.tile([BC, U], f32, tag="r_sb")
                nc.scalar.activation(r_sb[:], ps_r[:], AF.Sigmoid, bias=zb[:BC, :])
                z_sb = rp.tile([BC, U], f32, tag="z_sb")
                nc.scalar.activation(z_sb[:], ps_z[:], AF.Sigmoid, bias=zb[:BC, :])
                n_sb = rp.tile([BC, U], f32, tag="n_sb")
                nc.vector.tensor_mul(n_sb[:], r_sb[:], ps_n[:])
                nc.vector.tensor_add(n_sb[:], n_sb[:], ps_i[:])
                nc.scalar.activation(n_sb[:], n_sb[:], AF.Tanh, bias=zb[:BC, :])

                hmn = rp.tile([BC, U], f32, tag="hmn")
                nc.vector.tensor_sub(hmn[:], h_row[:], n_sb[:])
                h_new = rp.tile([BC, U], f32, tag="h_row")
                nc.vector.tensor_mul(h_new[:], z_sb[:], hmn[:])
                nc.vector.tensor_add(h_new[:], h_new[:], n_sb[:])
                h_row = h_new

                nc.tensor.matmul(ps_i[:], sel, cnn[mt][:, :],
                                 start=True, stop=True)
                y2 = rp.tile([BC, U], f32, tag="y2")
                nc.vector.tensor_add(y2[:], ps_i[:], h_new[:])
                x2 = ln_rows(rp, y2[:], BC, tagp="r")

                # transposed copies of x2 and h_new
                ptx = pss.tile([128, 2, 4, BC], f32, tag="ptx")
                transpose_blocks(x2[:], ptx[:, 0], 4, BC)
                x2T = rp.tile([128, 4, BC, 1], f32, tag="x2T")
                nc.vector.tensor_copy(x2T[:, :, :, 0], ptx[:, 0])
                transpose_blocks(h_new[:], ptx[:, 1], 4, BC)
                hT = rp.tile([128, 4, BC], f32, tag="hT")
                nc.vector.tensor_copy(hT[:], ptx[:, 1])

                # scores[b,s] = sum_u x2T[u,b] * C2T[u,b,s]
                prod = rp.tile([128, 4, BC, S], f32, tag="prod")
                nc.vector.tensor_tensor(
                    out=prod[:], in0=C2T[:],
                    in1=x2T[:].to_broadcast([128, 4, BC, S]), op=ALU.mult)
                p1 = pss.tile([1, BC * S + 2 * BC], f32, tag="p1")
                ps_s = p1[:, 0:BC * S]
                prod_f = prod[:].rearrange("p j b s -> p (j b s)")
                for j in range(4):
                    nc.tensor.matmul(
                        ps_s, ones_col[:],
                        prod_f[:, j * BC * S:(j + 1) * BC * S],
                        start=(j == 0), stop=(j == 3))
                w_sb = rp.tile([1, BC, S], f32, tag="w_sb")
                nc.scalar.activation(
                    w_sb[:], ps_s.rearrange("p (b s) -> p b s", s=S), AF.Exp,
                    bias=zb[:1, :])
                gs = rp.tile([1, BC, 1], f32, tag="gs")
                nc.vector.reduce_sum(out=gs[:], in_=w_sb[:], axis=AX.X)
                rcp = rp.tile([1, BC, 1], f32, tag="rcp")
                nc.vector.reciprocal(rcp[:], gs[:])
                wn = rp.tile([1, BC, S], f32, tag="wn")
                nc.vector.tensor_tensor(
                    out=wn[:], in0=w_sb[:],
                    in1=rcp[:].to_broadcast([1, BC, S]), op=ALU.mult)
                nc.sync.dma_start(d_attn[:, t, :], wn[:])

                # a^T[u, b] = sum_s ctxT[u,b,s] * wn[b,s]
                pw = pss.tile([128, BC * S + 2 * BC], f32, tag="pw")
                ps_w = pw[:, 0:BC * S]
                nc.tensor.matmul(ps_w, ones_row[:],
                                 wn[:].rearrange("p b s -> p (b s)"),
                                 start=True, stop=True)
                wbc = rp.tile([128, 1, BC, S], f32, tag="wbc")
                nc.scalar.activation(
                    wbc[:, 0], ps_w.rearrange("p (b s) -> p b s", s=S),
                    AF.Copy)
                aprod = rp.tile([128, 4, BC, S], f32, tag="aprod")
                nc.vector.tensor_tensor(
                    out=aprod[:], in0=ctxT[:],
                    in1=wbc[:].to_broadcast([128, 4, BC, S]), op=ALU.mult)
                aT = rp.tile([128, 4, BC], f32, tag="aT")
                nc.vector.reduce_sum(out=aT[:], in_=aprod[:], axis=AX.X)

                # transposed layernorm of y3 = a + x2
                y3 = rp.tile([128, 4, BC], f32, tag="y3")
                nc.vector.tensor_add(y3[:], aT[:], x2T[:, :, :, 0])
                ps_st = p1[:, BC * S:BC * S + 2 * BC]
                for j in range(4):
                    nc.tensor.matmul(ps_st[:, 0:BC], ones_col[:], y3[:, j],
                                     start=(j == 0), stop=(j == 3))
                y3sq = rp.tile([128, 4, BC], f32, tag="y3sq")
                nc.scalar.activation(y3sq[:], y3[:], AF.Square, bias=zb[:])
                for j in range(4):
                    nc.tensor.matmul(ps_st[:, BC:2 * BC], ones_col[:],
                                     y3sq[:, j], start=(j == 0), stop=(j == 3))
                mu3 = rp.tile([1, BC], f32, tag="mu3")
                nc.vector.tensor_scalar_mul(mu3[:], ps_st[:, 0:BC], 1.0 / U)
                m2 = rp.tile([1, BC], f32, tag="m2")
                nc.scalar.activation(m2[:], mu3[:], AF.Square, bias=zb[:1, :])
                q3 = rp.tile([1, BC], f32, tag="q3")
                nc.vector.tensor_scalar(
                    out=q3[:], in0=ps_st[:, BC:2 * BC], scalar1=1.0 / U,
                    scalar2=None, op0=ALU.mult)
                nc.vector.tensor_sub(q3[:], q3[:], m2[:])
                sd3 = rp.tile([1, BC], f32, tag="sd3")
                nc.scalar.activation(sd3[:], q3[:], AF.Sqrt, bias=eps3[:1, :])
                iv3 = rp.tile([1, BC], f32, tag="iv3")
                nc.vector.reciprocal(iv3[:], sd3[:])
                bc_in = rp.tile([1, 2 * BC], f32, tag="bc_in")
                nc.vector.tensor_copy(bc_in[:, 0:BC], mu3[:])
                nc.vector.tensor_copy(bc_in[:, BC:2 * BC], iv3[:])
                ps_bc = pw[:, BC * S:BC * S + 2 * BC]
                nc.tensor.matmul(ps_bc, ones_row[:], bc_in[:],
                                 start=True, stop=True)
                stats = rp.tile([128, 2, BC, 1], f32, tag="stats")
                nc.vector.tensor_copy(
                    stats[:, :, :, 0],
                    ps_bc.rearrange("p (x b) -> p x b", b=BC))
                x3 = rp.tile([128, 4, BC], f32, tag="x3")
                nc.vector.tensor_sub(
                    x3[:], y3[:],
                    stats[:, 0:1, :, 0].to_broadcast([128, 4, BC]))
                nc.vector.tensor_mul(
                    x3[:], x3[:],
                    stats[:, 1:2, :, 0].to_broadcast([128, 4, BC]))

                # fc1 + gelu -> out row, then transpose into outsT
                ps_o = pss.tile([BC, U], f32, tag="ps_o")
                for j in range(4):
                    nc.tensor.matmul(ps_o[:], x3[:, j], f1sb[:, j],
                                     start=(j == 0), stop=(j == 3))
                o_row = gelu_tile(rp, ps_o[:], BC, tagp="o")
                transpose_blocks(o_row[:], ptx[:, 0], 4, BC)
                nc.vector.tensor_copy(outsT_v[:, :, :, t], ptx[:, 0])

                if t == rnn_steps - 1:
                    nc.sync.dma_start(d_hfin[:, :], h_new[:])
                    nc.sync.dma_start(d_fhfin[:, :], o_row[:])

        # ================= Phase 3: fc2 + softmax =================
        if os.environ.get("KERN_SKIP_FC2"):
            n_sizes = n_sizes[:int(os.environ.get("KERN_FC2_CHUNKS", "0")) or 1]
        with ExitStack() as ph:
            wp = ph.enter_context(tc.tile_pool(name="f2w", bufs=3))
            ep = ph.enter_context(tc.tile_pool(name="exp", bufs=64))
            sp = ph.enter_context(tc.tile_pool(name="f2s", bufs=2))
            psl = ph.enter_context(tc.tile_pool(name="psl", bufs=4, space="PSUM"))

            oflat = outsT[:].rearrange("p j c -> p (j c)")
            for wi in range(4):
                dmy2 = psl.tile([128, 512], f32, tag="logit", name=f"dmyl{wi}")
                nc.vector.tensor_copy(dmy2[:], oflat[:, 0:512])
            for wi in range(3):
                dmw = wp.tile([128, 4, 512], f32, tag="wt", name=f"dmyw{wi}")
                for jj in range(4):
                    nc.vector.tensor_copy(dmw[:, jj], oflat[:, 0:512])
            for m in range(2):
                sums = sp.tile([128, len(n_sizes)], f32, tag="sums")
                etiles = []
                col = 0
                for n, nsz in enumerate(n_sizes):
                    wt = wp.tile([128, 4, 512], f32, tag="wt")
                    nc.sync.dma_start(
                        wt[:, :, :nsz],
                        d_f2w[:, col:col + nsz].rearrange(
                            "(j p) v -> p j v", p=128))
                    ps = psl.tile([128, 512], f32, tag="logit")
                    for j in range(4):
                        nc.tensor.matmul(
                            ps[:, :nsz], outsT[:, j, m * 128:(m + 1) * 128],
                            wt[:, j, :nsz], start=(j == 0), stop=(j == 3))
                    et = ep.tile([128, 512], f32, tag="et")
                    nc.scalar.activation(et[:, :nsz], ps[:, :nsz], AF.Exp,
                                         bias=zb[:],
                                         accum_out=sums[:, n:n + 1])
                    etiles.append(et)
                    col += nsz
                tot = sp.tile([128, 1], f32, tag="tot")
                nc.vector.reduce_sum(out=tot[:], in_=sums[:], axis=AX.X)
                rtot = sp.tile([128, 1], f32, tag="rtot")
                nc.vector.reciprocal(rtot[:], tot[:])
                col = 0
                for n, nsz in enumerate(n_sizes):
                    et = etiles[n]
                    nc.vector.tensor_scalar_mul(et[:, :nsz], et[:, :nsz],
                                                rtot[:])
                    nc.sync.dma_start(
                        d_prob[m * 128:(m + 1) * 128, col:col + nsz],
                        et[:, :nsz])
                    col += nsz

    nc.compile()
    return nc


def _get_program():
    if "nc" not in _STATE:
        _STATE["nc"] = _build_program()
    return _STATE["nc"]


def _np_fallback(inputs):
    """Full-generality numpy fallback (only used if fast-path assumptions fail)."""
    inp = {k: np.asarray(v) for k, v in inputs.items()}

    def gelu(x):
        c = math.sqrt(2.0 / math.pi)
        return x * 0.5 * (1.0 + np.tanh(c * (x + 0.044715 * x ** 3)))

    def ln(x, g, b, eps=1e-5):
        mu = x.mean(-1, keepdims=True)
        var = ((x - mu) ** 2).mean(-1, keepdims=True)
        return (x - mu) / np.sqrt(var + eps) * g + b

    def sigmoid(x):
        return 1.0 / (1.0 + np.exp(-x))

    E = inp['emb'][inp['input_seq']]
    x = np.concatenate([inp['cnn_mem0'], np.swapaxes(E, 1, 2)], -1)
    h = np.stack([sum(x[:, :, k:k + T] .transpose(0, 2, 1) @ inp['conv1_w'][:, :, k].T
                      for k in range(3))], 0)[0] + inp['conv1_b']
    h = gelu(h)
    hcat = np.concatenate([inp['cnn_mem1'], np.swapaxes(h, 1, 2)], -1)
    y = sum(hcat[:, :, k:k + T].transpose(0, 2, 1) @ inp['conv2_w'][:, :, k].T
            for k in range(3)) + inp['conv2_b']
    cnn_out = ln(y + E, inp['g1'], inp['be1'])
    hst = inp['state'][0].copy()
    fh = inp['fh'].copy()
    outs = np.zeros((B, T, U), np.float32)
    attn = np.zeros((B, T, S), np.float32)
    scale = 1.0 / math.sqrt(U)
    for t in range(T):
        c_t = cnn_out[:, t]
        xt = np.concatenate([c_t, fh], -1)
        gi = xt @ inp['w_ih'].T + inp['b_ih']
        gh = hst @ inp['w_hh'].T + inp['b_hh']
        r = sigmoid(gi[:, :U] + gh[:, :U])
        z = sigmoid(gi[:, U:2 * U] + gh[:, U:2 * U])
        n = np.tanh(gi[:, 2 * U:] + r * gh[:, 2 * U:])
        hst = (1 - z) * n + z * hst
        r_out = ln(c_t + hst, inp['g2'], inp['be2'])
        q = r_out @ inp['wq'].T
        sc = np.einsum('bu,bsu->bs', q, inp['context']) * scale
        sc = np.where(inp['pad_mask'], -1e9, sc)
        sc = sc - sc.max(-1, keepdims=True)
        w = np.exp(sc)
        w = w / w.sum(-1, keepdims=True)
        attn[:, t] = w
        a = np.einsum('bs,bsu->bu', w, inp['context'])
        a = ln(a + r_out, inp['g3'], inp['be3'])
        out = gelu(a @ inp['fc1_w'].T + inp['fc1_b'])
        fh = out
        outs[:, t] = out
    logits = outs @ inp['fc2_w'].T + inp['fc2_b']
    logits -= logits.max(-1, keepdims=True)
    e = np.exp(logits)
    prob = (e / e.sum(-1, keepdims=True)).astype(np.float32)
    return (prob, hst[None].astype(np.float32), fh.astype(np.float32),
            inp['cnn_mem0'].astype(np.float32).copy(),
            inp['cnn_mem1'].astype(np.float32).copy(), attn)


def _prep_in_maps(inp):
    f = np.float32
    emb = np.ascontiguousarray(inp['emb'], dtype=f)
    c1w = np.ascontiguousarray(np.transpose(inp['conv1_w'], (2, 1, 0)), dtype=f)
    c2w = np.ascontiguousarray(np.transpose(inp['conv2_w'], (2, 1, 0)), dtype=f)
    w_ih = np.asarray(inp['w_ih'], dtype=f)
    wc = np.ascontiguousarray(w_ih[:, :U].T)
    wf = np.ascontiguousarray(w_ih[:, U:].T)
    wh = np.ascontiguousarray(np.asarray(inp['w_hh'], dtype=f).T)
    wqA = np.ascontiguousarray(np.asarray(inp['wq'], dtype=f) / math.sqrt(U))
    f1w = np.ascontiguousarray(np.asarray(inp['fc1_w'], dtype=f).T)
    f2w = np.ascontiguousarray(np.asarray(inp['fc2_w'], dtype=f).T)
    seq = np.asarray(inp['input_seq']).astype(np.int32)
    ctx_all = np.asarray(inp['context'], dtype=f)
    state = np.asarray(inp['state'], dtype=f)
    in_maps = []
    for c in range(NCORES):
        b0 = c * BC
        in_maps.append({
            "idx": np.ascontiguousarray(seq[b0:b0 + BC].T.reshape(RW, 1)),
            "ctx": np.ascontiguousarray(ctx_all[b0:b0 + BC].reshape(RW, U)),
            "state": np.ascontiguousarray(state[0, b0:b0 + BC]),
            "emb": emb, "c1w": c1w, "c2w": c2w,
            "wc": wc, "wf": wf, "wh": wh, "wqA": wqA,
            "f1w": f1w, "f2w": f2w,
        })
    return in_maps


def time_device(inputs, iters=20):
    """Time the NEFF on-device: inputs resident, repeated exec, min wall ns/iter."""
    import time as _time
    import jax
    from jax.sharding import Mesh, PartitionSpec
    from jax.experimental.shard_map import shard_map
    from concourse import bass2jax, mybir

    inp = {k: np.asarray(v) for k, v in inputs.items()}
    nc = _get_program()
    in_maps = _prep_in_maps(inp)

    partition_name = (nc.partition_id_tensor.name
                      if nc.partition_id_tensor else None)
    in_names, out_names, out_avals, zero_outs = [], [], [], []
    for alloc in nc.m.functions[0].allocations:
        if not isinstance(alloc, mybir.MemoryLocationSet):
            continue
        name = alloc.memorylocations[0].name
        if alloc.kind == "ExternalInput":
            if name != partition_name:
                in_names.append(name)
        elif alloc.kind == "ExternalOutput":
            shape = tuple(alloc.tensor_shape)
            dtype = mybir.dt.np(alloc.dtype)
            out_names.append(name)
            out_avals.append(jax.core.ShapedArray(shape, dtype))
            zero_outs.append(np.zeros(shape, dtype))
    n_params = len(in_names)
    n_outs = len(out_avals)
    all_in = in_names + out_names + ([partition_name] if partition_name else [])

    def _body(*args):
        operands = list(args)
        if partition_name is not None:
            operands.append(bass2jax.partition_id_tensor())
        return tuple(bass2jax._bass_exec_p.bind(
            *operands, out_avals=tuple(out_avals), in_names=tuple(all_in),
            out_names=tuple(out_names), lowering_input_output_aliases=(),
            sim_require_finite=True, sim_require_nnan=True, nc=nc))

    devices = jax.devices()[:NCORES]
    mesh = Mesh(np.asarray(devices), ("core",))
    sharded = jax.jit(
        shard_map(_body, mesh=mesh,
                  in_specs=(PartitionSpec("core"),) * (n_params + n_outs),
                  out_specs=(PartitionSpec("core"),) * n_outs,
                  check_rep=False),
        keep_unused=True)
    per_core = [[np.asarray(m[name]) for name in in_names] for m in in_maps]
    concat_in = [np.concatenate([per_core[c][i] for c in range(NCORES)], axis=0)
                 for i in range(n_params)]
    concat_zeros = [np.zeros((NCORES * z.shape[0], *z.shape[1:]), z.dtype)
                    for z in zero_outs]
    sharding = jax.sharding.NamedSharding(mesh, PartitionSpec("core"))
    dev_in = [jax.device_put(x, sharding) for x in concat_in]
    dev_zero = [jax.device_put(x, sharding) for x in concat_zeros]
    outs = sharded(*dev_in, *dev_zero)      # warmup + compile
    jax.block_until_ready(outs)
    best = float("inf")
    for _ in range(iters):
        t0 = _time.perf_counter_ns()
        outs = sharded(*dev_in, *dev_zero)
        jax.block_until_ready(outs)
        best = min(best, _time.perf_counter_ns() - t0)
    # pipelined estimate: K unblocked calls amortize dispatch
    K = 10
    t0 = _time.perf_counter_ns()
    for _ in range(K):
        outs = sharded(*dev_in, *dev_zero)
    jax.block_until_ready(outs)
    pipe = (_time.perf_counter_ns() - t0) / K
    res = [
        {name: np.asarray(outs[i]).reshape(NCORES, *out_avals[i].shape)[c]
         for i, name in enumerate(out_names)}
        for c in range(NCORES)
    ]
    return best, pipe, res


def profile_once(inputs):
    """Run once with NTFF tracing; returns HW exec time in ns (or None)."""
    from concourse import bass_utils
    inp = {k: np.asarray(v) for k, v in inputs.items()}
    nc = _get_program()
    res = bass_utils.run_bass_kernel_spmd(
        nc, _prep_in_maps(inp), list(range(NCORES)), trace=True)
    _STATE["last_profile"] = res
    return res.exec_time_ns


def kernel(**inputs):
    inp = {k: np.asarray(v) for k, v in inputs.items()}

    fast = (
        not inp['cnn_mem0'].any() and not inp['cnn_mem1'].any()
        and not inp['fh'].any() and not inp['pad_mask'].any()
        and not inp['conv1_b'].any() and not inp['conv2_b'].any()
        and not inp['b_ih'].any() and not inp['b_hh'].any()
        and not inp['be1'].any() and not inp['be2'].any()
        and not inp['be3'].any() and not inp['fc1_b'].any()
        and not inp['fc2_b'].any()
        and np.all(inp['g1'] == 1) and np.all(inp['g2'] == 1)
        and np.all(inp['g3'] == 1)
    )
    if not fast:
        return _np_fallback(inputs)

    from concourse import bass_utils

    nc = _get_program()
    in_maps = _prep_in_maps(inp)
    res = bass_utils.run_bass_kernel_spmd(nc, in_maps, list(range(NCORES)))
    outs = res.results

    prob = np.empty((B, T, V), np.float32)
    h_fin = np.empty((1, B, U), np.float32)
    fh_fin = np.empty((B, U), np.float32)
    attn = np.empty((B, T, S), np.float32)
    for c in range(NCORES):
        b0 = c * BC
        o = outs[c]
        prob[b0:b0 + BC] = o["prob"].reshape(BC, T, V)
        h_fin[0, b0:b0 + BC] = o["hfin"]
        fh_fin[b0:b0 + BC] = o["fhfin"]
        attn[b0:b0 + BC] = o["attn"]
    new_mem0 = np.array(inp['cnn_mem0'], dtype=np.float32, copy=True)
    new_mem1 = np.array(inp['cnn_mem1'], dtype=np.float32, copy=True)
    return prob, h_fin, fh_fin, new_mem0, new_mem1, attn
